# revision 1
# baseline (speedup 1.0000x reference)
"""Sparse (sliding-window) GQA attention prefill kernel for 8 Trainium2 cores.

Sharding: TP=4 over KV heads (2 KV heads + 10 Q heads per core) x DP=2 over
batch. Core c: batch = c // 4, shard q4 = c % 4.

Device program (SPMD, identical on all cores; per-core data via inputs):
  A: per 512-wide s-tile: q/k projections in transposed layout ([head_dim,
     seq]) and the V projection (natural layout) off the same x tiles, so x
     loads once; RMS-norm sum-of-squares via Square + ones-matmul; per-s-tile
     AllReduce of the norm partials (hidden under A's compute). Rope chains
     (r = raw^-0.5 via one Ln/Exp pass; norm constants folded into
     host-prescaled bf16 cos/sin tables; batched in-place rope over all local
     heads) are emitted two s-tiles behind their AllReduce; the last two
     chains land inside phase B, before their slices are touched.
  B: g-outer banded attention: for each 2-query-tile s-range, all head-pair
     chunks: scoresT = K-tile.T @ qT per j-pair into one 2-bank PSUM tile,
     mask add, one paired exp, ones128-matmul denominator (lands
     pre-broadcast across partitions), P^T @ V accumulation. Software
     pipeline runs ACROSS group boundaries (dn/at lag the scores by one
     pair), so neither the exp latency nor a group drain ever idles the PE.
     Divide on evacuation via reciprocal_approx_fast. attnT ships via one
     AllGather per 512-wide s-range (4 total, ~40us fixed CC cost each),
     fired mid-B so phase C never waits on a collective.
  C: out = attnT.T @ wo col-shard; wo resident in three column panels with
     the smallest prefetched into wv's dead buffer during B; lhs slabs load
     whole (1KB-contiguous descriptors) in block-need order.
"""

import sys
import numpy as np

for _p in ("/opt/trn_rl_repo", "/root/.axon_site/_ro/trn_rl_repo"):
    if _p not in sys.path:
        sys.path.insert(0, _p)

import ml_dtypes

import concourse.bass as bass
import concourse.tile as tile
from concourse import bacc, mybir
from concourse import bass_utils

F32 = mybir.dt.float32
BF16 = mybir.dt.bfloat16
BF16_NP = ml_dtypes.bfloat16
AF = mybir.ActivationFunctionType
ALU = mybir.AluOpType


class Cfg:
    def __init__(self, S=2048, DIM=5120, HQ=40, HKV=8, TP=4, DP=2, SW=1024,
                 MSCALE=1.2079441541679836, EPS=1e-6):
        self.S, self.DIM, self.HQ, self.HKV = S, DIM, HQ, HKV
        self.TP, self.DP, self.SW = TP, DP, SW
        self.MSCALE, self.EPS = MSCALE, EPS
        self.D = 128
        self.NC = TP * DP
        self.HQL = HQ // TP          # local q heads
        self.KVL = HKV // TP         # local kv heads
        self.REP = HQ // HKV
        self.KC = DIM // 128         # contraction chunks
        self.NT = S // 128           # seq tiles
        self.G = self.NT // 2        # 2-query-tile groups
        self.WD = SW // 128          # window in tiles
        self.COLS = DIM // TP        # output column shard
        self.HCL = self.HQL + self.KVL  # projection chains with transposed out
        self.NST = S // 512          # 512-wide s-tiles (phase A1)
        self.NST2 = S // 256         # 256-wide s-tiles (phase A2)
        self.CQ = self.D ** -0.5 * MSCALE
        assert self.WD >= 2 and self.NT > self.WD + 1 and self.NT % 2 == 0
        self.groups = [[b * TP + r for r in range(TP)] for b in range(DP)]


def head_chunks(C):
    """Per-kv head pair chunks: [(kv, [h0,h1]), (kv, [h2,h3]), (kv, [h4])...]"""
    out = []
    per = C.HQL // C.KVL
    for kv in range(C.KVL):
        hs = list(range(kv * per, (kv + 1) * per))
        i = 0
        while i < len(hs):
            out.append((kv, hs[i:i + 2]))
            i += 2
    return out


def attention_tile_kernel(tc, C, io):
    nc = tc.nc
    S, KC, HQL, KVL, NT, G, WD = C.S, C.KC, C.HQL, C.KVL, C.NT, C.G, C.WD
    H2 = S // 2
    xT16, wqkv, wv_in, wo_in = io["xT16"], io["wqkv"], io["wv_in"], io["wo_in"]
    tabqc_in, tabqs_in = io["tabqc"], io["tabqs"]
    tabkc_in, tabks_in = io["tabkc"], io["tabks"]
    masks_in, wnorm_in = io["masks"], io["wnorm"]
    out_sh = io["out_sh"]
    chunks = head_chunks(C)

    from contextlib import ExitStack
    ctx = ExitStack()
    with ctx:
        singles = ctx.enter_context(tc.tile_pool(name="singles", bufs=1))
        dramcc = ctx.enter_context(tc.tile_pool(name="dramcc", bufs=1, space="DRAM"))

        ones16 = singles.tile([128, 1], BF16)
        nc.vector.memset(ones16[:], 1.0)
        ones128 = singles.tile([128, 128], BF16)
        nc.vector.memset(ones128[:], 1.0)
        wnorm_sb = singles.tile([128, HQL + KVL], F32)
        nc.sync.dma_start(wnorm_sb[:], wnorm_in[:])

        cc_nins = [dramcc.tile([1, 1024], F32, name=f"ccni{st}")
                   for st in range(C.NST)]
        cc_nouts = [dramcc.tile([1, 1024], F32, name=f"ccno{st}")
                    for st in range(C.NST)]
        # one gather per 512-wide s-range (phase B runs g-outer), so phase C
        # consumes s-slabs progressively and never waits on the last
        # collective; 4 gathers keeps the ~40us fixed CC cost per collective
        # well under phase B's span
        NGA = G // 2
        cc_ains = [dramcc.tile([HQL, 128, 512], BF16, name=f"ccag{gi}")
                   for gi in range(NGA)]
        cc_aouts = [dramcc.tile([C.TP, HQL, 128, 512], BF16,
                                name=f"ccaog{gi}") for gi in range(NGA)]

        wvp = ctx.enter_context(tc.tile_pool(name="wvp", bufs=1))
        wv_sb = wvp.tile([128, KC, KVL * 128], BF16)
        nc.gpsimd.dma_start(wv_sb[:], wv_in[:])

        with (
            tc.tile_pool(name="xqp", bufs=1) as xq_pool,
            tc.tile_pool(name="xkp", bufs=1) as xk_pool,
            tc.tile_pool(name="vp", bufs=1) as v_pool,
            tc.tile_pool(name="tabsQ", bufs=1) as tabs_q,
            tc.tile_pool(name="rowsp", bufs=1) as rowsp,
            tc.tile_pool(name="ropep", bufs=1) as ropep,
        ):
            xq_sb = xq_pool.tile([128, HQL, S], BF16)
            xk_sb = xk_pool.tile([128, KVL, S], BF16)
            v_sb = v_pool.tile([128, NT, KVL, 128], BF16)

            # ---- phase A: q/k projections (transposed out) + norm
            # partials + V projection, all per 512-wide s-tile. V reuses the
            # same xt tiles as stationary, so x is loaded exactly once.
            # Rope chains are emitted with a 2-tile lag so each slice's
            # AllReduce has completed long before its chain runs; slices 0-1
            # rope on the vector engine during A itself.
            KCH = KC // 2
            with (
                tc.tile_pool(name="xt1", bufs=2) as xt1,
                tc.tile_pool(name="wst", bufs=2) as wst,
                tc.tile_pool(name="sqp", bufs=3) as sqp,
                tc.tile_pool(name="trow", bufs=1) as trow,
                tc.tile_pool(name="psA", bufs=3, space="PSUM") as psA,
                tc.tile_pool(name="psN", bufs=1, space="PSUM") as psN,
                tc.tile_pool(name="psV", bufs=3, space="PSUM") as psV,
            ):

                def emit_rchain(st):
                    sl = slice(st * 512, (st + 1) * 512)
                    # r = raw^-0.5 for both rows in one Ln/Exp pass (norm
                    # constants are folded into the host-prescaled tables)
                    rowraw = rowsp.tile([1, 1024], F32, tag="rowraw")
                    nc.gpsimd.dma_start(rowraw[:], cc_nouts[st][:])
                    rb = rowsp.tile([128, 1024], F32, tag="rb")
                    nc.gpsimd.partition_broadcast(rb[:], rowraw[:])
                    nc.scalar.activation(rb[:], rb[:], AF.Ln)
                    nc.scalar.activation(rb[:], rb[:], AF.Exp, scale=-0.5)
                    cosq = tabs_q.tile([128, 512], BF16, tag="cosq")
                    sinq = tabs_q.tile([128, 512], BF16, tag="sinq")
                    cosk = tabs_q.tile([128, 512], BF16, tag="cosk")
                    sink = tabs_q.tile([128, 512], BF16, tag="sink")
                    nc.gpsimd.dma_start(cosk[:], tabkc_in[:, sl])
                    nc.gpsimd.dma_start(sink[:], tabks_in[:, sl])
                    nc.gpsimd.dma_start(cosq[:], tabqc_in[:, sl])
                    nc.gpsimd.dma_start(sinq[:], tabqs_in[:, sl])
                    nc.vector.tensor_mul(cosk[:], cosk[:], rb[:, 512:1024])
                    nc.vector.tensor_mul(sink[:], sink[:], rb[:, 512:1024])
                    nc.vector.tensor_mul(cosq[:], cosq[:], rb[:, 0:512])
                    nc.vector.tensor_mul(sinq[:], sinq[:], rb[:, 0:512])
                    rotk = ropep.tile([128, KVL, 512], BF16, tag="rotk")
                    rotq = ropep.tile([128, HQL, 512], BF16, tag="rotq")
                    nc.gpsimd.dma_start(rotk[0:64], xk_sb[64:128, :, sl])
                    nc.gpsimd.dma_start(rotk[64:128], xk_sb[0:64, :, sl])
                    nc.gpsimd.dma_start(rotq[0:64], xq_sb[64:128, :, sl])
                    nc.gpsimd.dma_start(rotq[64:128], xq_sb[0:64, :, sl])
                    # fully in-place on vector: rot *= sin; x *= cos; x += rot
                    nc.vector.tensor_mul(
                        rotk[:], rotk[:],
                        sink[:, None, :].to_broadcast((128, KVL, 512)))
                    nc.vector.tensor_mul(
                        xk_sb[:, :, sl], xk_sb[:, :, sl],
                        cosk[:, None, :].to_broadcast((128, KVL, 512)))
                    nc.vector.tensor_add(xk_sb[:, :, sl], xk_sb[:, :, sl],
                                         rotk[:])
                    nc.vector.tensor_mul(
                        rotq[:], rotq[:],
                        sinq[:, None, :].to_broadcast((128, HQL, 512)))
                    nc.vector.tensor_mul(
                        xq_sb[:, :, sl], xq_sb[:, :, sl],
                        cosq[:, None, :].to_broadcast((128, HQL, 512)))
                    nc.vector.tensor_add(xq_sb[:, :, sl], xq_sb[:, :, sl],
                                         rotq[:])

                for st in range(C.NST):
                    s0 = st * 512
                    xt_a = xt1.tile([128, KCH, 512], BF16, tag="xta")
                    xt_b = xt1.tile([128, KCH, 512], BF16, tag="xtb")
                    nc.sync.dma_start(
                        xt_a[:],
                        xT16[:KCH, :, s0:s0 + 512].rearrange("kc p s -> p kc s"))
                    nc.sync.dma_start(
                        xt_b[:],
                        xT16[KCH:, :, s0:s0 + 512].rearrange("kc p s -> p kc s"))
                    ps_nq = psN.tile([128, 512], F32, tag="nq")
                    ps_nk = psN.tile([128, 512], F32, tag="nk")
                    for hc in range(C.HCL):
                        w_sb = wst.tile([128, KC, 128], BF16, tag="w")
                        nc.sync.dma_start(w_sb[:], wqkv[hc])
                        ps = psA.tile([128, 512], F32, tag="proj")
                        for kc in range(KC):
                            xsrc = xt_a if kc < KCH else xt_b
                            nc.tensor.matmul(ps[:], w_sb[:, kc, :],
                                             xsrc[:, kc % KCH, :],
                                             start=(kc == 0), stop=(kc == KC - 1))
                        if hc < HQL:
                            dest = xq_sb[:, hc, s0:s0 + 512]
                        else:
                            dest = xk_sb[:, hc - HQL, s0:s0 + 512]
                        nc.vector.tensor_scalar_mul(dest, ps[:],
                                                    wnorm_sb[:, hc:hc + 1])
                        sq = sqp.tile([128, 512], BF16, tag="sq")
                        nc.scalar.activation(sq[:], ps[:], AF.Square)
                        tgt = ps_nq if hc < HQL else ps_nk
                        first = (hc == 0) or (hc == HQL)
                        last = (hc == HQL - 1) or (hc == C.HCL - 1)
                        nc.tensor.matmul(tgt[:], ones128[:], sq[:],
                                         start=first, stop=last)
                    rq_t = trow.tile([1, 512], F32, tag="rq")
                    rk_t = trow.tile([1, 512], F32, tag="rk")
                    nc.vector.tensor_copy(rq_t[:], ps_nq[0:1, :])
                    nc.vector.tensor_copy(rk_t[:], ps_nk[0:1, :])
                    nc.sync.dma_start(cc_nins[st][0:1, 0:512], rq_t[:])
                    nc.sync.dma_start(cc_nins[st][0:1, 512:1024], rk_t[:])
                    # per-s-tile AllReduce of norm partials: overlaps A compute
                    nc.gpsimd.collective_compute(
                        "AllReduce", ALU.add, replica_groups=C.groups,
                        ins=[cc_nins[st].opt()], outs=[cc_nouts[st].opt()])
                    # V projection for this s-tile, x chunks as stationary
                    for tc4 in range(4):
                        tt = st * 4 + tc4
                        psv = psV.tile([128, KVL * 128], F32, tag="v")
                        for kc in range(KC):
                            xsrc = xt_a if kc < KCH else xt_b
                            nc.tensor.matmul(
                                psv[:],
                                xsrc[:, kc % KCH,
                                     tc4 * 128:(tc4 + 1) * 128],
                                wv_sb[:, kc, :],
                                start=(kc == 0), stop=(kc == KC - 1))
                        nc.scalar.copy(v_sb[:, tt, :, :], psv[:])
                    if st >= 2:
                        emit_rchain(st - 2)


            # prefetch the small wo panel into wv's buffer (same shape;
            # wv is dead after A) so phase C starts without waiting on it
            wo0_sb = wvp.tile([128, C.HQ, 256], BF16, tag="wv_sb")
            nc.sync.dma_start(wo0_sb[:], wo_in[:, :, 1024:1280])

            # ---- phase B: banded attention, software-pipelined --------------
            with (
                tc.tile_pool(name="attnp", bufs=1) as attnp,
                tc.tile_pool(name="maskp", bufs=1) as maskp,
                tc.tile_pool(name="expp", bufs=6) as expp,
                tc.tile_pool(name="bmisc", bufs=4) as bmisc,
                tc.tile_pool(name="psSC", bufs=2, space="PSUM") as psSC,
                tc.tile_pool(name="psAT", bufs=2, space="PSUM") as psAT,
                tc.tile_pool(name="psDN", bufs=2, space="PSUM") as psDN,
            ):
                attnT = attnp.tile([128, HQL, S], BF16)
                masks_sb = maskp.tile([128, 4, 256], F32)
                nc.sync.dma_start(masks_sb[:], masks_in[:])
                off2m = {0: 0, 1: 1, WD: 2, WD + 1: 3}

                pend = []  # (ex slice, j, group-state) awaiting dn/at

                def evac(stt):
                    w, h0, g = stt["w"], stt["h0"], stt["g"]
                    den_b = bmisc.tile([128, 512], F32, tag="denb")
                    nc.vector.reciprocal_approx_fast(
                        out=den_b[:, :w * 256], in_=stt["dn"][:, :w * 256])
                    nc.vector.tensor_mul(
                        attnT[:, h0:h0 + w, g * 256:(g + 1) * 256],
                        stt["at"][:, :w * 256].rearrange(
                            "p (w s) -> p w s", w=w),
                        den_b[:, :w * 256].rearrange(
                            "p (w s) -> p w s", w=w))

                def drain_one():
                    ex_ap, j, stt = pend.pop(0)
                    w, kv = stt["w"], stt["kv"]
                    nc.tensor.matmul(stt["dn"][:, :w * 256], ones128[:],
                                     ex_ap, start=(j == stt["jlo"]),
                                     stop=(j == stt["jhi"]))
                    nc.tensor.matmul(stt["at"][:, :w * 256],
                                     v_sb[:, j, kv, :],
                                     ex_ap, start=(j == stt["jlo"]),
                                     stop=(j == stt["jhi"]))
                    if j == stt["jhi"]:
                        evac(stt)

                for g in range(G):
                    jlo, jhi = max(0, 2 * g - WD), 2 * g + 1
                    npairs = (jhi - jlo + 1) // 2
                    for ci, (kv, hs) in enumerate(chunks):
                        w = len(hs)
                        h0 = hs[0]
                        stt = {
                            "w": w, "kv": kv, "h0": h0, "g": g,
                            "jlo": jlo, "jhi": jhi,
                            # ones128 stationary -> denominator lands
                            # pre-broadcast across all partitions
                            "at": psAT.tile([128, 512], F32, tag="at",
                                            name="ps_at"),
                            "dn": psDN.tile([128, 512], F32, tag="dn",
                                            name="ps_dn"),
                        }
                        for p in range(npairs):
                            j0 = jlo + 2 * p
                            ps2 = psSC.tile([128, 1024], F32, tag="sc")
                            for dj in range(2):
                                j = j0 + dj
                                o = dj * 512
                                nc.tensor.matmul(
                                    ps2[:, o:o + w * 256].rearrange(
                                        "p (w s) -> p w s", w=w),
                                    xk_sb[:, kv, j * 128:(j + 1) * 128],
                                    xq_sb[:, h0:h0 + w,
                                          g * 256:(g + 1) * 256],
                                    start=True, stop=True)
                            # consume older pairs (also across group
                            # boundaries) so the drain's exp latency is
                            # never exposed at a boundary
                            while len(pend) > 2:
                                drain_one()
                            for dj in range(2):
                                j = j0 + dj
                                m = off2m.get(jhi - j)
                                if m is not None:
                                    o = dj * 512
                                    nc.vector.tensor_add(
                                        ps2[:, o:o + w * 256].rearrange(
                                            "p (w s) -> p w s", w=w),
                                        ps2[:, o:o + w * 256].rearrange(
                                            "p (w s) -> p w s", w=w),
                                        masks_sb[:, m, None,
                                                 :].to_broadcast(
                                            (128, w, 256)))
                            ex2 = expp.tile([128, 1024], BF16, tag="ex")
                            nc.scalar.activation(
                                ex2.rearrange("p (j s) -> p j s",
                                              j=2)[:, :, :w * 256],
                                ps2.rearrange("p (j s) -> p j s",
                                              j=2)[:, :, :w * 256],
                                AF.Exp)
                            pend.append((ex2[:, 0:w * 256], j0, stt))
                            pend.append((ex2[:, 512:512 + w * 256],
                                         j0 + 1, stt))
                    if g % 2 == 1:
                        # drain so the slab DMA is emitted after the evacs
                        # it reads, then ship + gather
                        while pend:
                            drain_one()
                        gi = g // 2
                        nc.sync.dma_start(
                            cc_ains[gi].rearrange("h p s -> p h s"),
                            attnT[:, :, (g - 1) * 256:(g + 1) * 256])
                        nc.gpsimd.collective_compute(
                            "AllGather", ALU.bypass,
                            replica_groups=C.groups,
                            ins=[cc_ains[gi].opt()],
                            outs=[cc_aouts[gi].opt()])
                    if g == 1 or g == 3:
                        # last slices' rope chains: their AllReduces land
                        # after A ends, and B only touches slice 2 from g=4
                        # and slice 3 from g=6 onward.
                        emit_rchain(C.NST - 2 + g // 2)
                while pend:
                    drain_one()

        # ---- phase C: output projection ----------------------------------
        # wo resident in three column panels; the small panel and the first
        # lhs sub-slice load first so the PE starts ~12us after B. lhs slabs
        # load whole (1KB-contiguous segments -> cheap descriptors).
        with (
            tc.tile_pool(name="wop", bufs=1) as wop,
            tc.tile_pool(name="lhsp", bufs=2) as lhsp,
            tc.tile_pool(name="outp", bufs=4) as outp,
            tc.tile_pool(name="psO", bufs=4, space="PSUM") as psO,
        ):
            col_ts = [(1024, 256), (0, 512), (512, 512)]
            wo_ps = [wo0_sb] + [
                wop.tile([128, C.HQ, 512], BF16, tag=f"wop{ci}",
                         name=f"wop{ci}") for ci in (1, 2)]

            def c_block(blk, lhs, cis):
                for ci in cis:
                    c0, wdt = col_ts[ci]
                    for sbl in range(4):
                        sb = blk * 4 + sbl
                        off = sbl * 128
                        ps_o = psO.tile([128, 512], F32, tag="o")
                        for slot in range(C.HQ):
                            nc.tensor.matmul(
                                ps_o[:, :wdt],
                                lhs[:, slot, off:off + 128],
                                wo_ps[ci][:, slot, :],
                                start=(slot == 0),
                                stop=(slot == C.HQ - 1))
                        oro = outp.tile([128, 512], F32, tag="oro")
                        nc.vector.tensor_copy(oro[:, :wdt], ps_o[:, :wdt])
                        nc.sync.dma_start(
                            out_sh[sb * 128:(sb + 1) * 128, c0:c0 + wdt],
                            oro[:, :wdt])

            # all big loads serialized in need-order on the gpsimd queue,
            # which is empty after B's last gather: each transfer then runs
            # at near-full bandwidth and finishes before its consumer,
            # instead of all of them flooding the bus at the B->C seam
            lhs0 = lhsp.tile([128, C.HQ, 512], BF16, tag="lhs")
            nc.gpsimd.dma_start(
                lhs0[:],
                cc_aouts[0].rearrange("r h p s -> p (r h) s"))
            nc.gpsimd.dma_start(wo_ps[1][:], wo_in[:, :, 0:512])
            nc.gpsimd.dma_start(wo_ps[2][:], wo_in[:, :, 512:1024])
            c_block(0, lhs0, [0])
            lhs_n = {}
            for blk in range(1, NGA):
                lhs_n[blk] = lhsp.tile([128, C.HQ, 512], BF16, tag="lhs",
                                       name=f"lhsn{blk}")
                nc.gpsimd.dma_start(
                    lhs_n[blk][:],
                    cc_aouts[blk].rearrange("r h p s -> p (r h) s"))
            c_block(0, lhs0, [1, 2])
            for blk in range(1, NGA):
                c_block(blk, lhs_n[blk], [0, 1, 2])


def build_program(C):
    nc = bacc.Bacc("TRN2", target_bir_lowering=False, debug=False,
                   num_devices=C.NC)
    io = {
        "xT16": nc.dram_tensor("xT16", [C.KC, 128, C.S], BF16, kind="ExternalInput").ap(),
        "wqkv": nc.dram_tensor("wqkv", [C.HCL, 128, C.KC, 128], BF16,
                               kind="ExternalInput").ap(),
        "wv_in": nc.dram_tensor("wv_in", [128, C.KC, C.KVL * 128], BF16,
                                kind="ExternalInput").ap(),
        "wo_in": nc.dram_tensor("wo_in", [128, C.HQ, C.COLS], BF16,
                                kind="ExternalInput").ap(),
        "tabqc": nc.dram_tensor("tabqc", [128, C.S], BF16, kind="ExternalInput").ap(),
        "tabqs": nc.dram_tensor("tabqs", [128, C.S], BF16, kind="ExternalInput").ap(),
        "tabkc": nc.dram_tensor("tabkc", [128, C.S], BF16, kind="ExternalInput").ap(),
        "tabks": nc.dram_tensor("tabks", [128, C.S], BF16, kind="ExternalInput").ap(),
        "masks": nc.dram_tensor("masks", [128, 4, 256], F32, kind="ExternalInput").ap(),
        "wnorm": nc.dram_tensor("wnorm", [128, C.HQL + C.KVL], F32,
                                kind="ExternalInput").ap(),
        "out_sh": nc.dram_tensor("out_sh", [C.S, C.COLS], F32,
                                 kind="ExternalOutput").ap(),
    }
    with tile.TileContext(nc) as tc:
        attention_tile_kernel(tc, C, io)
    nc.compile()
    return nc


def make_masks(mask_np, C):
    """4 mask tiles [t,s-pair] for offsets {0,1,WD,WD+1}; returns [128,4,256] f32."""
    S, WD, SW = C.S, C.WD, C.SW
    I0 = WD + 1

    def tileT(d):
        i, j = I0, I0 - d
        if 0 <= j < C.NT:
            blk = np.array(mask_np[i * 128:(i + 1) * 128, j * 128:(j + 1) * 128],
                           dtype=np.float64)
        else:
            blk = np.full((128, 128), -np.inf)
        s_idx = np.arange(128)[:, None]
        t_idx = np.arange(128)[None, :]
        dist = 128 * d + s_idx - t_idx
        blk = np.where(dist > SW, -np.inf, blk)
        return np.maximum(blk.T, -1e30).astype(np.float32)   # [t, s]

    tiles = []
    for off in (0, 1, WD, WD + 1):
        dl, dr = off - 1, off
        tiles.append(np.concatenate([tileT(dl), tileT(dr)], axis=1))
    return np.ascontiguousarray(np.stack(tiles, axis=1))      # [128, 4, 256]


def make_core_inputs(inputs, C):
    x = np.asarray(inputs["x"], dtype=np.float32)
    wq = np.asarray(inputs["wq"], dtype=np.float32)
    wk = np.asarray(inputs["wk"], dtype=np.float32)
    wv = np.asarray(inputs["wv"], dtype=np.float32)
    wo = np.asarray(inputs["wo"], dtype=np.float32)
    qw = np.asarray(inputs["q_norm_weight"], dtype=np.float32)
    kw = np.asarray(inputs["k_norm_weight"], dtype=np.float32)
    ch = np.asarray(inputs["cos_half"], dtype=np.float32)
    sh = np.asarray(inputs["sin_half"], dtype=np.float32)
    mask = np.asarray(inputs["mask"], dtype=np.float32)
    assert int(inputs.get("start_pos", 0) or 0) == 0

    cosT = np.ascontiguousarray(np.concatenate([ch.T, ch.T], axis=0))
    sinT = np.ascontiguousarray(np.concatenate([-sh.T, sh.T], axis=0))
    # norm constants folded into the tables: r = raw^-0.5 on device
    cstq = C.CQ * np.sqrt(C.DIM)
    cstk = np.sqrt(C.HKV * 128.0)
    tabqc = (cosT * cstq).astype(BF16_NP)
    tabqs = (sinT * cstq).astype(BF16_NP)
    tabkc = (cosT * cstk).astype(BF16_NP)
    tabks = (sinT * cstk).astype(BF16_NP)
    masks = make_masks(mask, C)
    KC, HQL, KVL = C.KC, C.HQL, C.KVL

    xT_cache = {}
    for b in range(C.DP):
        xT_cache[b] = np.ascontiguousarray(x[b].T).astype(BF16_NP).reshape(
            C.KC, 128, C.S)
    in_maps = []
    for c in range(C.NC):
        b, q4 = c // C.TP, c % C.TP
        x16 = xT_cache[b]
        wq_s = wq[:, 128 * HQL * q4:128 * HQL * (q4 + 1)]
        wk_s = wk[:, 128 * KVL * q4:128 * KVL * (q4 + 1)]
        wv_s = wv[:, 128 * KVL * q4:128 * KVL * (q4 + 1)]
        wqk = np.concatenate([wq_s, wk_s], axis=1).astype(BF16_NP)
        # [HCL, 128, KC, 128]: per chain, contraction-partition-major
        wqkv_pre = np.ascontiguousarray(
            wqk.reshape(KC, 128, C.HCL, 128).transpose(2, 1, 0, 3))
        wv_pre = np.ascontiguousarray(
            wv_s.astype(BF16_NP).reshape(KC, 128, KVL * 128).transpose(1, 0, 2))
        wo_s = wo[:, C.COLS * q4:C.COLS * (q4 + 1)].astype(BF16_NP)
        wo_pre = np.ascontiguousarray(
            wo_s.reshape(C.HQ, 128, C.COLS).transpose(1, 0, 2))
        wn = np.zeros((128, HQL + KVL), dtype=np.float32)
        for hc in range(HQL):
            g = HQL * q4 + hc
            wn[:, hc] = qw[128 * g:128 * (g + 1)]
        for j in range(KVL):
            g = KVL * q4 + j
            wn[:, HQL + j] = kw[128 * g:128 * (g + 1)]
        in_maps.append({"xT16": x16, "wqkv": wqkv_pre, "wv_in": wv_pre,
                        "wo_in": wo_pre, "tabqc": tabqc, "tabqs": tabqs,
                        "tabkc": tabkc, "tabks": tabks,
                        "masks": masks, "wnorm": wn})
    return in_maps


_CACHED = {}


def run(inputs, C=None, trace=False, stitch=None, trace_cores=None):
    C = C or Cfg()
    key = (C.S, C.DIM, C.HQ, C.HKV, C.TP, C.DP, C.SW)
    if key not in _CACHED:
        _CACHED[key] = build_program(C)
    nc = _CACHED[key]
    in_maps = make_core_inputs(inputs, C)
    if stitch is None:
        stitch = trace
    if trace and trace_cores is None:
        trace_cores = list(range(C.NC))
    res = bass_utils.run_bass_kernel_spmd(
        nc, in_maps, core_ids=list(range(C.NC)), trace=trace,
        stitch_traces=stitch, trace_cores=trace_cores if trace else None)
    out = np.empty((C.DP, C.S, C.DIM), dtype=np.float32)
    for c in range(C.NC):
        b, q4 = c // C.TP, c % C.TP
        out[b, :, C.COLS * q4:C.COLS * (q4 + 1)] = res.results[c]["out_sh"]
    return out, res


def kernel(**inputs) -> np.ndarray:
    out, _ = run(inputs)
    return out



# revision 15
# speedup vs baseline: 1.0423x; 1.0423x over previous
"""Sparse (sliding-window) GQA attention prefill kernel for 8 Trainium2 cores.

Sharding: TP=4 over KV heads (2 KV heads + 10 Q heads per core) x DP=2 over
batch. Core c: batch = c // 4, shard q4 = c % 4.

Device program (SPMD, identical on all cores; per-core data via inputs):
  A: per 512-wide s-tile: q/k projections in transposed layout ([head_dim,
     seq]) and the V projection (natural layout) off the same x tiles, so x
     loads once; RMS-norm sum-of-squares via Square + ones-matmul; per-s-tile
     AllReduce of the norm partials (hidden under A's compute). Rope chains
     (r = raw^-0.5 via one Ln/Exp pass; norm constants folded into
     host-prescaled bf16 cos/sin tables; batched in-place rope over all local
     heads) are emitted two s-tiles behind their AllReduce; the last two
     chains land inside phase B, before their slices are touched.
  B: g-outer banded attention: for each 2-query-tile s-range, all head-pair
     chunks: scoresT = K-tile.T @ qT per j-pair into one 2-bank PSUM tile,
     mask add, one paired exp, ones128-matmul denominator (lands
     pre-broadcast across partitions), P^T @ V accumulation. Software
     pipeline runs ACROSS group boundaries (dn/at lag the scores by one
     pair), so neither the exp latency nor a group drain ever idles the PE.
     Divide on evacuation via reciprocal_approx_fast. attnT ships via one
     AllGather per 512-wide s-range (4 total, ~40us fixed CC cost each),
     fired mid-B so phase C never waits on a collective.
  C: out = attnT.T @ wo col-shard; wo resident in three column panels with
     the smallest prefetched into wv's dead buffer during B; lhs slabs load
     whole (1KB-contiguous descriptors) in block-need order.
"""

import sys
import numpy as np

for _p in ("/opt/trn_rl_repo", "/root/.axon_site/_ro/trn_rl_repo"):
    if _p not in sys.path:
        sys.path.insert(0, _p)

import ml_dtypes

import concourse.bass as bass
import concourse.tile as tile
from concourse import bacc, mybir
from concourse import bass_utils

F32 = mybir.dt.float32
BF16 = mybir.dt.bfloat16
BF16_NP = ml_dtypes.bfloat16
AF = mybir.ActivationFunctionType
ALU = mybir.AluOpType


class Cfg:
    def __init__(self, S=2048, DIM=5120, HQ=40, HKV=8, TP=4, DP=2, SW=1024,
                 MSCALE=1.2079441541679836, EPS=1e-6):
        self.S, self.DIM, self.HQ, self.HKV = S, DIM, HQ, HKV
        self.TP, self.DP, self.SW = TP, DP, SW
        self.MSCALE, self.EPS = MSCALE, EPS
        self.D = 128
        self.NC = TP * DP
        self.HQL = HQ // TP          # local q heads
        self.KVL = HKV // TP         # local kv heads
        self.REP = HQ // HKV
        self.KC = DIM // 128         # contraction chunks
        self.NT = S // 128           # seq tiles
        self.G = self.NT // 2        # 2-query-tile groups
        self.WD = SW // 128          # window in tiles
        self.COLS = DIM // TP        # output column shard
        self.HCL = self.HQL + self.KVL  # projection chains with transposed out
        self.NST = S // 512          # 512-wide s-tiles (phase A1)
        self.NST2 = S // 256         # 256-wide s-tiles (phase A2)
        self.CQ = self.D ** -0.5 * MSCALE
        assert self.WD >= 2 and self.NT > self.WD + 1 and self.NT % 2 == 0
        self.groups = [[b * TP + r for r in range(TP)] for b in range(DP)]


def head_chunks(C):
    """Per-kv head pair chunks: [(kv, [h0,h1]), (kv, [h2,h3]), (kv, [h4])...]"""
    out = []
    per = C.HQL // C.KVL
    for kv in range(C.KVL):
        hs = list(range(kv * per, (kv + 1) * per))
        i = 0
        while i < len(hs):
            out.append((kv, hs[i:i + 2]))
            i += 2
    return out


def attention_tile_kernel(tc, C, io):
    nc = tc.nc
    S, KC, HQL, KVL, NT, G, WD = C.S, C.KC, C.HQL, C.KVL, C.NT, C.G, C.WD
    H2 = S // 2
    xT16, wqkv, wv_in, wo_p = io["xT16"], io["wqkv"], io["wv_in"], io["wo_p"]
    tabqc_in, tabqs_in = io["tabqc"], io["tabqs"]
    tabkc_in, tabks_in = io["tabkc"], io["tabks"]
    masks_in, wnorm_in = io["masks"], io["wnorm"]
    out_sh = io["out_sh"]
    chunks = head_chunks(C)

    from contextlib import ExitStack
    ctx = ExitStack()
    with ctx:
        singles = ctx.enter_context(tc.tile_pool(name="singles", bufs=1))
        dramcc = ctx.enter_context(tc.tile_pool(name="dramcc", bufs=1, space="DRAM"))

        ones16 = singles.tile([128, 1], BF16)
        nc.vector.memset(ones16[:], 1.0)
        ones128 = singles.tile([128, 128], BF16)
        nc.vector.memset(ones128[:], 1.0)
        wnorm_sb = singles.tile([128, HQL + KVL], F32)
        nc.sync.dma_start(wnorm_sb[:], wnorm_in[:])

        cc_nins = [dramcc.tile([1, 1024], F32, name=f"ccni{st}")
                   for st in range(C.NST)]
        cc_nouts = [dramcc.tile([1, 1024], F32, name=f"ccno{st}")
                    for st in range(C.NST)]
        # one gather per 512-wide s-range (phase B runs g-outer), so phase C
        # consumes s-slabs progressively and never waits on the last
        # collective; 4 gathers keeps the ~40us fixed CC cost per collective
        # well under phase B's span. p-major layout so slab stores and the
        # gathered lhs loads use long per-partition runs (few descriptors).
        NGA = G // 2
        cc_ains = [dramcc.tile([128, HQL, 512], BF16, name=f"ccag{gi}")
                   for gi in range(NGA)]
        cc_aouts = [dramcc.tile([C.TP, 128, HQL, 512], BF16,
                                name=f"ccaog{gi}") for gi in range(NGA)]

        wvp = ctx.enter_context(tc.tile_pool(name="wvp", bufs=1))
        wv_sb = wvp.tile([128, KC, KVL * 128], BF16)
        nc.gpsimd.dma_start(wv_sb[:], wv_in[:])

        with (
            tc.tile_pool(name="xqp", bufs=1) as xq_pool,
            tc.tile_pool(name="xkp", bufs=1) as xk_pool,
            tc.tile_pool(name="vp", bufs=1) as v_pool,
            tc.tile_pool(name="tabsQ", bufs=1) as tabs_q,
            tc.tile_pool(name="rowsp", bufs=1) as rowsp,
            tc.tile_pool(name="ropep", bufs=1) as ropep,
        ):
            xq_sb = xq_pool.tile([128, HQL, S], BF16)
            xk_sb = xk_pool.tile([128, KVL, S], BF16)
            v_sb = v_pool.tile([128, NT, KVL, 128], BF16)

            # ---- phase A: q/k projections (transposed out) + norm
            # partials + V projection, all per 512-wide s-tile. V reuses the
            # same xt tiles as stationary, so x is loaded exactly once.
            # Rope chains are emitted with a 2-tile lag so each slice's
            # AllReduce has completed long before its chain runs; slices 0-1
            # rope on the vector engine during A itself.
            KCH = KC // 2
            with (
                tc.tile_pool(name="xt1", bufs=2) as xt1,
                tc.tile_pool(name="wst", bufs=2) as wst,
                tc.tile_pool(name="sqp", bufs=3) as sqp,
                tc.tile_pool(name="trow", bufs=1) as trow,
                tc.tile_pool(name="psA", bufs=3, space="PSUM") as psA,
                tc.tile_pool(name="psN", bufs=1, space="PSUM") as psN,
                tc.tile_pool(name="psV", bufs=3, space="PSUM") as psV,
            ):

                def emit_rchain(st):
                    sl = slice(st * 512, (st + 1) * 512)
                    # r = raw^-0.5 for both rows in one Ln/Exp pass (norm
                    # constants are folded into the host-prescaled tables)
                    rowraw = rowsp.tile([1, 1024], F32, tag="rowraw")
                    nc.gpsimd.dma_start(rowraw[:], cc_nouts[st][:])
                    rb = rowsp.tile([128, 1024], F32, tag="rb")
                    nc.gpsimd.partition_broadcast(rb[:], rowraw[:])
                    nc.scalar.activation(rb[:], rb[:], AF.Ln)
                    nc.scalar.activation(rb[:], rb[:], AF.Exp, scale=-0.5)
                    cosq = tabs_q.tile([128, 512], BF16, tag="cosq")
                    sinq = tabs_q.tile([128, 512], BF16, tag="sinq")
                    cosk = tabs_q.tile([128, 512], BF16, tag="cosk")
                    sink = tabs_q.tile([128, 512], BF16, tag="sink")
                    nc.gpsimd.dma_start(cosk[:], tabkc_in[:, sl])
                    nc.gpsimd.dma_start(sink[:], tabks_in[:, sl])
                    nc.gpsimd.dma_start(cosq[:], tabqc_in[:, sl])
                    nc.gpsimd.dma_start(sinq[:], tabqs_in[:, sl])
                    nc.vector.tensor_mul(cosk[:], cosk[:], rb[:, 512:1024])
                    nc.vector.tensor_mul(sink[:], sink[:], rb[:, 512:1024])
                    nc.vector.tensor_mul(cosq[:], cosq[:], rb[:, 0:512])
                    nc.vector.tensor_mul(sinq[:], sinq[:], rb[:, 0:512])
                    rotk = ropep.tile([128, KVL, 512], BF16, tag="rotk")
                    rotq = ropep.tile([128, HQL, 512], BF16, tag="rotq")
                    nc.gpsimd.dma_start(rotk[0:64], xk_sb[64:128, :, sl])
                    nc.gpsimd.dma_start(rotk[64:128], xk_sb[0:64, :, sl])
                    nc.gpsimd.dma_start(rotq[0:64], xq_sb[64:128, :, sl])
                    nc.gpsimd.dma_start(rotq[64:128], xq_sb[0:64, :, sl])
                    # fully in-place on vector: rot *= sin; x *= cos; x += rot
                    nc.vector.tensor_mul(
                        rotk[:], rotk[:],
                        sink[:, None, :].to_broadcast((128, KVL, 512)))
                    nc.vector.tensor_mul(
                        xk_sb[:, :, sl], xk_sb[:, :, sl],
                        cosk[:, None, :].to_broadcast((128, KVL, 512)))
                    nc.vector.tensor_add(xk_sb[:, :, sl], xk_sb[:, :, sl],
                                         rotk[:])
                    nc.vector.tensor_mul(
                        rotq[:], rotq[:],
                        sinq[:, None, :].to_broadcast((128, HQL, 512)))
                    nc.vector.tensor_mul(
                        xq_sb[:, :, sl], xq_sb[:, :, sl],
                        cosq[:, None, :].to_broadcast((128, HQL, 512)))
                    nc.vector.tensor_add(xq_sb[:, :, sl], xq_sb[:, :, sl],
                                         rotq[:])

                def load_xt(st):
                    # p-major host layout: one ~20KB contiguous run per
                    # partition, so the whole 512-wide slab is ~256 cheap
                    # descriptors instead of 5120 1KB ones. On the scalar
                    # queue (nearly idle in A) with a one-tile lookahead so
                    # the transfer fully overlaps the previous tile's chains.
                    xt_a = xt1.tile([128, KCH, 512], BF16, tag="xta")
                    xt_b = xt1.tile([128, KCH, 512], BF16, tag="xtb")
                    nc.scalar.dma_start(xt_a[:], xT16[:, st, :KCH, :])
                    nc.scalar.dma_start(xt_b[:], xT16[:, st, KCH:, :])
                    return xt_a, xt_b

                nxt = load_xt(0)
                for st in range(C.NST):
                    s0 = st * 512
                    xt_a, xt_b = nxt
                    if st + 1 < C.NST:
                        nxt = load_xt(st + 1)
                    ps_nq = psN.tile([128, 512], F32, tag="nq")
                    ps_nk = psN.tile([128, 512], F32, tag="nk")
                    for hc in range(C.HCL):
                        w_sb = wst.tile([128, KC, 128], BF16, tag="w")
                        nc.sync.dma_start(w_sb[:], wqkv[hc])
                        ps = psA.tile([128, 512], F32, tag="proj")
                        for kc in range(KC):
                            xsrc = xt_a if kc < KCH else xt_b
                            nc.tensor.matmul(ps[:], w_sb[:, kc, :],
                                             xsrc[:, kc % KCH, :],
                                             start=(kc == 0), stop=(kc == KC - 1))
                        if hc < HQL:
                            dest = xq_sb[:, hc, s0:s0 + 512]
                        else:
                            dest = xk_sb[:, hc - HQL, s0:s0 + 512]
                        nc.vector.tensor_scalar_mul(dest, ps[:],
                                                    wnorm_sb[:, hc:hc + 1])
                        sq = sqp.tile([128, 512], BF16, tag="sq")
                        nc.scalar.activation(sq[:], ps[:], AF.Square)
                        tgt = ps_nq if hc < HQL else ps_nk
                        first = (hc == 0) or (hc == HQL)
                        last = (hc == HQL - 1) or (hc == C.HCL - 1)
                        nc.tensor.matmul(tgt[:], ones128[:], sq[:],
                                         start=first, stop=last)
                    rq_t = trow.tile([1, 512], F32, tag="rq")
                    rk_t = trow.tile([1, 512], F32, tag="rk")
                    nc.vector.tensor_copy(rq_t[:], ps_nq[0:1, :])
                    nc.vector.tensor_copy(rk_t[:], ps_nk[0:1, :])
                    nc.sync.dma_start(cc_nins[st][0:1, 0:512], rq_t[:])
                    nc.sync.dma_start(cc_nins[st][0:1, 512:1024], rk_t[:])
                    # per-s-tile AllReduce of norm partials: overlaps A compute
                    nc.gpsimd.collective_compute(
                        "AllReduce", ALU.add, replica_groups=C.groups,
                        ins=[cc_nins[st].opt()], outs=[cc_nouts[st].opt()])
                    # V projection for this s-tile, x chunks as stationary
                    for tc4 in range(4):
                        tt = st * 4 + tc4
                        psv = psV.tile([128, KVL * 128], F32, tag="v")
                        for kc in range(KC):
                            xsrc = xt_a if kc < KCH else xt_b
                            nc.tensor.matmul(
                                psv[:],
                                xsrc[:, kc % KCH,
                                     tc4 * 128:(tc4 + 1) * 128],
                                wv_sb[:, kc, :],
                                start=(kc == 0), stop=(kc == KC - 1))
                        nc.scalar.copy(v_sb[:, tt, :, :], psv[:])
                    if st >= 2:
                        emit_rchain(st - 2)


            # prefetch the small wo panel into wv's buffer (same shape;
            # wv is dead after A) so phase C starts without waiting on it
            wo0_sb = wvp.tile([128, C.HQ, 256], BF16, tag="wv_sb")
            nc.sync.dma_start(wo0_sb[:], wo_p[0][:])

            # ---- phase B: banded attention, software-pipelined --------------
            with (
                tc.tile_pool(name="attnp", bufs=1) as attnp,
                tc.tile_pool(name="maskp", bufs=1) as maskp,
                tc.tile_pool(name="expp", bufs=6) as expp,
                tc.tile_pool(name="bmisc", bufs=4) as bmisc,
                tc.tile_pool(name="psSC", bufs=2, space="PSUM") as psSC,
                tc.tile_pool(name="psAT", bufs=2, space="PSUM") as psAT,
                tc.tile_pool(name="psDN", bufs=2, space="PSUM") as psDN,
            ):
                # [128, gather-slab, head, 512]: per-partition-contiguous
                # slabs so each cc_ains store is one long run per partition
                attnT = attnp.tile([128, NGA, HQL, 512], BF16)
                masks_sb = maskp.tile([128, 4, 256], F32)
                nc.sync.dma_start(masks_sb[:], masks_in[:])
                off2m = {0: 0, 1: 1, WD: 2, WD + 1: 3}

                pend = []  # (ex slice, j, group-state) awaiting dn/at

                def evac(stt):
                    w, h0, g = stt["w"], stt["h0"], stt["g"]
                    den_b = bmisc.tile([128, 512], F32, tag="denb")
                    nc.vector.reciprocal_approx_fast(
                        out=den_b[:, :w * 256], in_=stt["dn"][:, :w * 256])
                    o = (g % 2) * 256
                    nc.vector.tensor_mul(
                        attnT[:, g // 2, h0:h0 + w, o:o + 256],
                        stt["at"][:, :w * 256].rearrange(
                            "p (w s) -> p w s", w=w),
                        den_b[:, :w * 256].rearrange(
                            "p (w s) -> p w s", w=w))

                def drain_one():
                    ex_ap, j, stt = pend.pop(0)
                    w, kv = stt["w"], stt["kv"]
                    nc.tensor.matmul(stt["dn"][:, :w * 256], ones128[:],
                                     ex_ap, start=(j == stt["jlo"]),
                                     stop=(j == stt["jhi"]))
                    nc.tensor.matmul(stt["at"][:, :w * 256],
                                     v_sb[:, j, kv, :],
                                     ex_ap, start=(j == stt["jlo"]),
                                     stop=(j == stt["jhi"]))
                    if j == stt["jhi"]:
                        evac(stt)

                for g in range(G):
                    jlo, jhi = max(0, 2 * g - WD), 2 * g + 1
                    npairs = (jhi - jlo + 1) // 2
                    for ci, (kv, hs) in enumerate(chunks):
                        w = len(hs)
                        h0 = hs[0]
                        stt = {
                            "w": w, "kv": kv, "h0": h0, "g": g,
                            "jlo": jlo, "jhi": jhi,
                            # ones128 stationary -> denominator lands
                            # pre-broadcast across all partitions
                            "at": psAT.tile([128, 512], F32, tag="at",
                                            name="ps_at"),
                            "dn": psDN.tile([128, 512], F32, tag="dn",
                                            name="ps_dn"),
                        }
                        for p in range(npairs):
                            j0 = jlo + 2 * p
                            ps2 = psSC.tile([128, 1024], F32, tag="sc")
                            for dj in range(2):
                                j = j0 + dj
                                o = dj * 512
                                nc.tensor.matmul(
                                    ps2[:, o:o + w * 256].rearrange(
                                        "p (w s) -> p w s", w=w),
                                    xk_sb[:, kv, j * 128:(j + 1) * 128],
                                    xq_sb[:, h0:h0 + w,
                                          g * 256:(g + 1) * 256],
                                    start=True, stop=True)
                            # consume older pairs (also across group
                            # boundaries) so the drain's exp latency is
                            # never exposed at a boundary
                            while len(pend) > 2:
                                drain_one()
                            for dj in range(2):
                                j = j0 + dj
                                m = off2m.get(jhi - j)
                                if m is not None:
                                    o = dj * 512
                                    nc.vector.tensor_add(
                                        ps2[:, o:o + w * 256].rearrange(
                                            "p (w s) -> p w s", w=w),
                                        ps2[:, o:o + w * 256].rearrange(
                                            "p (w s) -> p w s", w=w),
                                        masks_sb[:, m, None,
                                                 :].to_broadcast(
                                            (128, w, 256)))
                            ex2 = expp.tile([128, 1024], BF16, tag="ex")
                            nc.scalar.activation(
                                ex2.rearrange("p (j s) -> p j s",
                                              j=2)[:, :, :w * 256],
                                ps2.rearrange("p (j s) -> p j s",
                                              j=2)[:, :, :w * 256],
                                AF.Exp)
                            pend.append((ex2[:, 0:w * 256], j0, stt))
                            pend.append((ex2[:, 512:512 + w * 256],
                                         j0 + 1, stt))
                    if g % 2 == 1:
                        # drain so the slab DMA is emitted after the evacs
                        # it reads, then ship + gather
                        while pend:
                            drain_one()
                        gi = g // 2
                        nc.sync.dma_start(cc_ains[gi][:], attnT[:, gi, :, :])
                        nc.gpsimd.collective_compute(
                            "AllGather", ALU.bypass,
                            replica_groups=C.groups,
                            ins=[cc_ains[gi].opt()],
                            outs=[cc_aouts[gi].opt()])
                    if g == 1 or g == 3:
                        # last slices' rope chains: their AllReduces land
                        # after A ends, and B only touches slice 2 from g=4
                        # and slice 3 from g=6 onward.
                        emit_rchain(C.NST - 2 + g // 2)
                while pend:
                    drain_one()

        # ---- phase C: output projection ----------------------------------
        # wo resident in three column panels; the small panel and the first
        # lhs sub-slice load first so the PE starts ~12us after B. lhs slabs
        # load whole (1KB-contiguous segments -> cheap descriptors).
        with (
            tc.tile_pool(name="wop", bufs=1) as wop,
            tc.tile_pool(name="lhsp", bufs=2) as lhsp,
            tc.tile_pool(name="outp", bufs=4) as outp,
            tc.tile_pool(name="psO", bufs=4, space="PSUM") as psO,
        ):
            col_ts = [(1024, 256), (0, 512), (512, 512)]
            wo_ps = [wo0_sb] + [
                wop.tile([128, C.HQ, 512], BF16, tag=f"wop{ci}",
                         name=f"wop{ci}") for ci in (1, 2)]

            def c_block(blk, lhs, cis):
                for ci in cis:
                    c0, wdt = col_ts[ci]
                    for sbl in range(4):
                        sb = blk * 4 + sbl
                        off = sbl * 128
                        ps_o = psO.tile([128, 512], F32, tag="o")
                        for slot in range(C.HQ):
                            nc.tensor.matmul(
                                ps_o[:, :wdt],
                                lhs[:, slot, off:off + 128],
                                wo_ps[ci][:, slot, :],
                                start=(slot == 0),
                                stop=(slot == C.HQ - 1))
                        oro = outp.tile([128, 512], F32, tag="oro")
                        nc.vector.tensor_copy(oro[:, :wdt], ps_o[:, :wdt])
                        nc.sync.dma_start(
                            out_sh[sb * 128:(sb + 1) * 128, c0:c0 + wdt],
                            oro[:, :wdt])

            # all big loads in need-order on the scalar queue: it has no
            # collectives, so lhs0 doesn't serialize behind the last
            # AllGather (which completes only after the globally-slowest
            # core's phase B). Each transfer is p-major (10KB+ runs per
            # partition) so it streams at near-full bandwidth.
            lhs0 = lhsp.tile([128, C.HQ, 512], BF16, tag="lhs")
            nc.scalar.dma_start(
                lhs0[:].rearrange("p (r h) s -> p r h s", r=C.TP),
                cc_aouts[0].rearrange("r p h s -> p r h s"))
            nc.scalar.dma_start(wo_ps[1][:], wo_p[1][:])
            nc.scalar.dma_start(wo_ps[2][:], wo_p[2][:])
            c_block(0, lhs0, [0])
            lhs_n = {}
            for blk in range(1, NGA):
                lhs_n[blk] = lhsp.tile([128, C.HQ, 512], BF16, tag="lhs",
                                       name=f"lhsn{blk}")
                nc.scalar.dma_start(
                    lhs_n[blk][:].rearrange("p (r h) s -> p r h s", r=C.TP),
                    cc_aouts[blk].rearrange("r p h s -> p r h s"))
            c_block(0, lhs0, [1, 2])
            for blk in range(1, NGA):
                c_block(blk, lhs_n[blk], [0, 1, 2])


def build_program(C):
    nc = bacc.Bacc("TRN2", target_bir_lowering=False, debug=False,
                   num_devices=C.NC)
    io = {
        "xT16": nc.dram_tensor("xT16", [128, C.NST, C.KC, 512], BF16,
                               kind="ExternalInput").ap(),
        "wqkv": nc.dram_tensor("wqkv", [C.HCL, 128, C.KC, 128], BF16,
                               kind="ExternalInput").ap(),
        "wv_in": nc.dram_tensor("wv_in", [128, C.KC, C.KVL * 128], BF16,
                                kind="ExternalInput").ap(),
        "wo_p": [nc.dram_tensor(f"wo_p{ci}", [128, C.HQ, w], BF16,
                                kind="ExternalInput").ap()
                 for ci, w in ((0, 256), (1, 512), (2, 512))],
        "tabqc": nc.dram_tensor("tabqc", [128, C.S], BF16, kind="ExternalInput").ap(),
        "tabqs": nc.dram_tensor("tabqs", [128, C.S], BF16, kind="ExternalInput").ap(),
        "tabkc": nc.dram_tensor("tabkc", [128, C.S], BF16, kind="ExternalInput").ap(),
        "tabks": nc.dram_tensor("tabks", [128, C.S], BF16, kind="ExternalInput").ap(),
        "masks": nc.dram_tensor("masks", [128, 4, 256], F32, kind="ExternalInput").ap(),
        "wnorm": nc.dram_tensor("wnorm", [128, C.HQL + C.KVL], F32,
                                kind="ExternalInput").ap(),
        "out_sh": nc.dram_tensor("out_sh", [C.S, C.COLS], F32,
                                 kind="ExternalOutput").ap(),
    }
    with tile.TileContext(nc) as tc:
        attention_tile_kernel(tc, C, io)
    nc.compile()
    return nc


def make_masks(mask_np, C):
    """4 mask tiles [t,s-pair] for offsets {0,1,WD,WD+1}; returns [128,4,256] f32."""
    S, WD, SW = C.S, C.WD, C.SW
    I0 = WD + 1

    def tileT(d):
        i, j = I0, I0 - d
        if 0 <= j < C.NT:
            blk = np.array(mask_np[i * 128:(i + 1) * 128, j * 128:(j + 1) * 128],
                           dtype=np.float64)
        else:
            blk = np.full((128, 128), -np.inf)
        s_idx = np.arange(128)[:, None]
        t_idx = np.arange(128)[None, :]
        dist = 128 * d + s_idx - t_idx
        blk = np.where(dist > SW, -np.inf, blk)
        return np.maximum(blk.T, -1e30).astype(np.float32)   # [t, s]

    tiles = []
    for off in (0, 1, WD, WD + 1):
        dl, dr = off - 1, off
        tiles.append(np.concatenate([tileT(dl), tileT(dr)], axis=1))
    return np.ascontiguousarray(np.stack(tiles, axis=1))      # [128, 4, 256]


def make_core_inputs(inputs, C):
    x = np.asarray(inputs["x"], dtype=np.float32)
    wq = np.asarray(inputs["wq"], dtype=np.float32)
    wk = np.asarray(inputs["wk"], dtype=np.float32)
    wv = np.asarray(inputs["wv"], dtype=np.float32)
    wo = np.asarray(inputs["wo"], dtype=np.float32)
    qw = np.asarray(inputs["q_norm_weight"], dtype=np.float32)
    kw = np.asarray(inputs["k_norm_weight"], dtype=np.float32)
    ch = np.asarray(inputs["cos_half"], dtype=np.float32)
    sh = np.asarray(inputs["sin_half"], dtype=np.float32)
    mask = np.asarray(inputs["mask"], dtype=np.float32)
    assert int(inputs.get("start_pos", 0) or 0) == 0

    cosT = np.ascontiguousarray(np.concatenate([ch.T, ch.T], axis=0))
    sinT = np.ascontiguousarray(np.concatenate([-sh.T, sh.T], axis=0))
    # norm constants folded into the tables: r = raw^-0.5 on device
    cstq = C.CQ * np.sqrt(C.DIM)
    cstk = np.sqrt(C.HKV * 128.0)
    tabqc = (cosT * cstq).astype(BF16_NP)
    tabqs = (sinT * cstq).astype(BF16_NP)
    tabkc = (cosT * cstk).astype(BF16_NP)
    tabks = (sinT * cstk).astype(BF16_NP)
    masks = make_masks(mask, C)
    KC, HQL, KVL = C.KC, C.HQL, C.KVL

    xT_cache = {}
    for b in range(C.DP):
        # [128, NST, KC, 512]: p-major so each per-partition st-slab is one
        # contiguous KC*512*2 = 20KB run
        xT_cache[b] = np.ascontiguousarray(
            x[b].T.reshape(C.KC, 128, C.NST, 512).transpose(1, 2, 0, 3)
        ).astype(BF16_NP)
    in_maps = []
    for c in range(C.NC):
        b, q4 = c // C.TP, c % C.TP
        x16 = xT_cache[b]
        wq_s = wq[:, 128 * HQL * q4:128 * HQL * (q4 + 1)]
        wk_s = wk[:, 128 * KVL * q4:128 * KVL * (q4 + 1)]
        wv_s = wv[:, 128 * KVL * q4:128 * KVL * (q4 + 1)]
        wqk = np.concatenate([wq_s, wk_s], axis=1).astype(BF16_NP)
        # [HCL, 128, KC, 128]: per chain, contraction-partition-major
        wqkv_pre = np.ascontiguousarray(
            wqk.reshape(KC, 128, C.HCL, 128).transpose(2, 1, 0, 3))
        wv_pre = np.ascontiguousarray(
            wv_s.astype(BF16_NP).reshape(KC, 128, KVL * 128).transpose(1, 0, 2))
        wo_s = wo[:, C.COLS * q4:C.COLS * (q4 + 1)].astype(BF16_NP)
        wo_r = wo_s.reshape(C.HQ, 128, C.COLS)
        # three p-major column panels, matching kernel need-order
        wo_pre = {f"wo_p{ci}": np.ascontiguousarray(
                      wo_r[:, :, c0:c0 + w].transpose(1, 0, 2))
                  for ci, (c0, w) in enumerate(
                      ((1024, 256), (0, 512), (512, 512)))}
        wn = np.zeros((128, HQL + KVL), dtype=np.float32)
        for hc in range(HQL):
            g = HQL * q4 + hc
            wn[:, hc] = qw[128 * g:128 * (g + 1)]
        for j in range(KVL):
            g = KVL * q4 + j
            wn[:, HQL + j] = kw[128 * g:128 * (g + 1)]
        in_maps.append({"xT16": x16, "wqkv": wqkv_pre, "wv_in": wv_pre,
                        "tabqc": tabqc, "tabqs": tabqs,
                        "tabkc": tabkc, "tabks": tabks,
                        "masks": masks, "wnorm": wn, **wo_pre})
    return in_maps


_CACHED = {}


def run(inputs, C=None, trace=False, stitch=None, trace_cores=None):
    C = C or Cfg()
    key = (C.S, C.DIM, C.HQ, C.HKV, C.TP, C.DP, C.SW)
    if key not in _CACHED:
        _CACHED[key] = build_program(C)
    nc = _CACHED[key]
    in_maps = make_core_inputs(inputs, C)
    if stitch is None:
        stitch = trace
    if trace and trace_cores is None:
        trace_cores = list(range(C.NC))
    res = bass_utils.run_bass_kernel_spmd(
        nc, in_maps, core_ids=list(range(C.NC)), trace=trace,
        stitch_traces=stitch, trace_cores=trace_cores if trace else None)
    out = np.empty((C.DP, C.S, C.DIM), dtype=np.float32)
    for c in range(C.NC):
        b, q4 = c // C.TP, c % C.TP
        out[b, :, C.COLS * q4:C.COLS * (q4 + 1)] = res.results[c]["out_sh"]
    return out, res


def kernel(**inputs) -> np.ndarray:
    out, _ = run(inputs)
    return out

